# revision 7
# baseline (speedup 1.0000x reference)
"""Causal self-attention TRN2 kernel.

Full module: x[4,2048,1024] @ W_qkv[1024,3072] -> heads(16,d=64) causal attn
-> @ W_proj[1024,1024].

Sharding: 8 cores = 4 batches x 2 head-groups (8 heads each), tensor-parallel
over heads. Each core computes q/k/v for its 8 heads, causal attention, and a
partial projection (row-sharded W_proj). The two partials per batch are summed
on the host (no on-device collectives).

Per-core kernel layout (all f32, matmuls in f32r):
  phase 1 (QKV):  qT/kT chunks [128, T] (feature-major, head pairs per chunk),
                  v65 token tiles [128, 8 heads, 64 V + 1 ones column]
  phase 2 (attn): scores^T [k,q] per 128-k-tile via row-packed pair matmuls,
                  exp on ACT (scale=1/8 folded in), causal mask multiply on
                  diagonal tiles, y^T = [V|1]^T @ P~ accumulated in PSUM
                  (row 64 = softmax denominators), divide via reciprocal +
                  K=1 matmul partition-broadcast
  phase 3 (proj): out[t,:] partial = yT^T @ wp, interleaved per q-chunk
"""

import numpy as np
from contextlib import ExitStack

import concourse.bass as bass
import concourse.tile as tile
from concourse import mybir, bacc
from concourse.bass_utils import run_bass_kernel_spmd

F32 = mybir.dt.float32
F32R = mybir.dt.float32r
EXP = mybir.ActivationFunctionType.Exp

B, T, C, H, D = 4, 2048, 1024, 16, 64
NCORES = 8
GROUPS = 2            # head groups (tensor-parallel dimension)
HPC = H // GROUPS     # heads per core = 8
FPC = HPC * D         # features per core = 512
SCALE = 1.0 / np.sqrt(D)


def build_nc(T=T, C=C, HPC=HPC):
    FPC = HPC * D
    NC = C // 128     # contraction chunks over C
    NT = T // 128     # token tiles (also k-tiles)
    NQ = T // 512     # query chunks
    NF = FPC // 128   # feature tiles = head pairs
    NN = max(C // 512, 1)  # proj output column chunks

    nc = bacc.Bacc("TRN2", debug=False)
    xT_d = nc.dram_tensor("xT", [C, T], F32R, kind="ExternalInput").ap()
    wq_d = nc.dram_tensor("wq", [C, FPC], F32R, kind="ExternalInput").ap()
    wk_d = nc.dram_tensor("wk", [C, FPC], F32R, kind="ExternalInput").ap()
    wv_d = nc.dram_tensor("wv", [C, FPC], F32R, kind="ExternalInput").ap()
    wp_d = nc.dram_tensor("wp", [FPC, C], F32R, kind="ExternalInput").ap()
    mk_d = nc.dram_tensor("masks", [4, 128, 512], F32R, kind="ExternalInput").ap()
    on_d = nc.dram_tensor("ones64", [1, 64], F32R, kind="ExternalInput").ap()
    ov_d = nc.dram_tensor("onesv", [128, 8, 1], F32R, kind="ExternalInput").ap()
    out_d = nc.dram_tensor("out", [T, C], F32, kind="ExternalOutput").ap()

    with tile.TileContext(nc) as tc, ExitStack() as ctx:
        # pools that span phases
        p_qt = ctx.enter_context(tc.tile_pool(name="p_qt", bufs=NF))
        p_kt = ctx.enter_context(tc.tile_pool(name="p_kt", bufs=NF))
        p_v65 = ctx.enter_context(tc.tile_pool(name="p_v65", bufs=NT))
        p_const = ctx.enter_context(tc.tile_pool(name="p_const", bufs=1))

        # ones row at partition 64 (for the K=1 broadcast matmul)
        ones_t = p_const.tile([65, 64], F32R, tag="ones")
        nc.sync.dma_start(out=ones_t[64:65, :], in_=on_d[:])

        qt = [p_qt.tile([128, T], F32R, tag="qt", name=f"qt{i}") for i in range(NF)]
        kt_ = [p_kt.tile([128, T], F32R, tag="kt", name=f"kt{i}") for i in range(NF)]
        v65 = [p_v65.tile([128, HPC, 65], F32R, tag="v65", name=f"v65_{i}") for i in range(NT)]

        # ---------------- phase 1: QKV projections ----------------
        with ExitStack() as p1:
            p_xt = p1.enter_context(tc.tile_pool(name="p_xt", bufs=NC))
            p_w = p1.enter_context(tc.tile_pool(name="p_w", bufs=NC))
            p_wv = p1.enter_context(tc.tile_pool(name="p_wv", bufs=NC))
            ps_1 = p1.enter_context(tc.tile_pool(name="ps_1", bufs=3, space="PSUM"))

            xt = []
            for c in range(NC):
                t_ = p_xt.tile([128, T], F32R, tag="xt")
                nc.sync.dma_start(out=t_[:], in_=xT_d[c * 128:(c + 1) * 128, :])
                xt.append(t_)
            wv_sb = []
            for c in range(NC):
                t_ = p_wv.tile([128, FPC], F32R, tag="wv")
                nc.sync.dma_start(out=t_[:], in_=wv_d[c * 128:(c + 1) * 128, :])
                wv_sb.append(t_)

            # qT / kT: out[feat, tok] = w.T @ x.T ; lhsT = w chunk, rhs = xT
            for w_d, dst, wtag in ((wq_d, qt, "wq"), (wk_d, kt_, "wk")):
                for f in range(NF):
                    wtiles = []
                    for c in range(NC):
                        wt = p_w.tile([128, 128], F32R, tag=wtag)
                        nc.sync.dma_start(
                            out=wt[:],
                            in_=w_d[c * 128:(c + 1) * 128, f * 128:(f + 1) * 128])
                        wtiles.append(wt)
                    for n in range(T // 512):
                        ps = ps_1.tile([128, 512], F32, tag="qkps")
                        for c in range(NC):
                            nc.tensor.matmul(
                                ps[:],
                                wtiles[c][:],
                                xt[c][:, n * 512:(n + 1) * 512],
                                start=(c == 0), stop=(c == NC - 1))
                        nc.scalar.copy(out=dst[f][:, n * 512:(n + 1) * 512], in_=ps[:])

            # v natural: out[tok, feat] = x @ wv ; lhsT = xT slice, rhs = wv
            for t in range(NT):
                ps = ps_1.tile([128, FPC], F32, tag="vps")
                for c in range(NC):
                    nc.tensor.matmul(
                        ps[:],
                        xt[c][:, t * 128:(t + 1) * 128],
                        wv_sb[c][:],
                        start=(c == 0), stop=(c == NC - 1))
                nc.vector.tensor_copy(
                    out=v65[t][:, :, 0:64],
                    in_=ps[:].rearrange("p (h d) -> p h d", h=HPC))
                nc.sync.dma_start(out=v65[t][:, :, 64:65], in_=ov_d[:, 0:HPC, :])

        # ---------------- phase 2: attention + projection ----------------
        with ExitStack() as p2:
            p_mask = p2.enter_context(tc.tile_pool(name="p_mask", bufs=4))
            p_yt = p2.enter_context(tc.tile_pool(name="p_yt", bufs=NF))
            p_pt = p2.enter_context(tc.tile_pool(name="p_pt", bufs=3))
            p_rec = p2.enter_context(tc.tile_pool(name="p_rec", bufs=2))
            p_ybt = p2.enter_context(tc.tile_pool(name="p_ybt", bufs=2))
            p_wp = p2.enter_context(tc.tile_pool(name="p_wp", bufs=NF))
            p_osb = p2.enter_context(tc.tile_pool(name="p_osb", bufs=2))
            ps_s = p2.enter_context(tc.tile_pool(name="ps_s", bufs=2, space="PSUM"))
            ps_y = p2.enter_context(tc.tile_pool(name="ps_y", bufs=1, space="PSUM"))
            ps_b = p2.enter_context(tc.tile_pool(name="ps_b", bufs=1, space="PSUM"))

            mk = []
            for d in range(4):
                mt = p_mask.tile([128, 512], F32R, tag="mask")
                nc.sync.dma_start(out=mt[:], in_=mk_d[d])
                mk.append(mt)
            wp_sb = []
            for cf in range(NF):
                wt = p_wp.tile([128, C], F32R, tag="wp")
                nc.sync.dma_start(out=wt[:], in_=wp_d[cf * 128:(cf + 1) * 128, :])
                wp_sb.append(wt)
            yt = [p_yt.tile([128, T], F32R, tag="yt", name=f"yt{i}") for i in range(NF)]

            for qc in range(NQ):
                qsl = slice(qc * 512, (qc + 1) * 512)
                for hp in range(NF):
                    nk = 4 * qc + 4
                    y_psA = ps_y.tile([65, 512], F32, tag="ya")
                    y_psB = ps_y.tile([65, 512], F32, tag="yb")
                    for kt in range(nk):
                        s_ps = ps_s.tile([128, 1024], F32, tag="s")
                        # scores^T = k_h @ q_h^T for the head pair, row-packed
                        nc.tensor.matmul(
                            s_ps[:, 0:512],
                            kt_[hp][0:64, kt * 128:(kt + 1) * 128],
                            qt[hp][0:64, qsl],
                            start=True, stop=True, tile_position=(0, 0))
                        nc.tensor.matmul(
                            s_ps[:, 512:1024],
                            kt_[hp][64:128, kt * 128:(kt + 1) * 128],
                            qt[hp][64:128, qsl],
                            start=True, stop=True, tile_position=(64, 0))
                        pt = p_pt.tile([128, 1024], F32R, tag="pt")
                        nc.scalar.activation(
                            out=pt[:], in_=s_ps[:], func=EXP, scale=float(SCALE))
                        if kt >= 4 * qc:
                            d = kt - 4 * qc
                            nc.vector.tensor_mul(pt[:, 0:512], pt[:, 0:512], mk[d][:])
                            nc.vector.tensor_mul(pt[:, 512:1024], pt[:, 512:1024], mk[d][:])
                        # y^T += [V|1]^T @ P~  (row 64 accumulates denominators)
                        nc.tensor.matmul(
                            y_psA[:],
                            v65[kt][:, 2 * hp, :],
                            pt[:, 0:512],
                            start=(kt == 0), stop=(kt == nk - 1))
                        nc.tensor.matmul(
                            y_psB[:],
                            v65[kt][:, 2 * hp + 1, :],
                            pt[:, 512:1024],
                            start=(kt == 0), stop=(kt == nk - 1))

                    # softmax division: recip of row 64, broadcast via K=1 matmul
                    recA = p_rec.tile([65, 512], F32R, tag="rec")
                    with nc.allow_low_precision("f32r softmax denom reciprocal"):
                        nc.vector.reciprocal(out=recA[64:65, :], in_=y_psA[64:65, :])
                    bcA = ps_b.tile([64, 512], F32, tag="bc")
                    nc.tensor.matmul(
                        bcA[:], ones_t[64:65, :],
                        recA[64:65, :],
                        start=True, stop=True, tile_position=(64, 0))
                    bcsA = p_ybt.tile([64, 512], F32, tag="bcs")
                    nc.vector.tensor_copy(out=bcsA[:], in_=bcA[:])
                    nc.vector.tensor_mul(yt[hp][0:64, qsl], y_psA[0:64, :], bcsA[:])

                    recB = p_rec.tile([65, 512], F32R, tag="rec")
                    with nc.allow_low_precision("f32r softmax denom reciprocal"):
                        nc.vector.reciprocal(out=recB[64:65, :], in_=y_psB[64:65, :])
                    bcB = ps_b.tile([64, 512], F32, tag="bc")
                    nc.tensor.matmul(
                        bcB[:], ones_t[64:65, :],
                        recB[64:65, :],
                        start=True, stop=True, tile_position=(64, 0))
                    bcsB = p_ybt.tile([64, 512], F32, tag="bcs")
                    nc.vector.tensor_copy(out=bcsB[:], in_=bcB[:])
                    ybt = p_ybt.tile([64, 512], F32R, tag="ybt")
                    nc.vector.tensor_mul(ybt[:], y_psB[0:64, :], bcsB[:])
                    nc.sync.dma_start(out=yt[hp][64:128, qsl], in_=ybt[:])

                # projection for this q-chunk's token tiles
                for t in range(4 * qc, 4 * qc + 4):
                    osb = p_osb.tile([128, C], F32, tag="osb")
                    for nn in range(NN):
                        npj = min(512, C)
                        pj = ps_b.tile([128, npj], F32, tag="pj")
                        for cf in range(NF):
                            nc.tensor.matmul(
                                pj[:],
                                yt[cf][:, t * 128:(t + 1) * 128],
                                wp_sb[cf][:, nn * npj:(nn + 1) * npj],
                                start=(cf == 0), stop=(cf == NF - 1))
                        nc.vector.tensor_copy(
                            out=osb[:, nn * npj:(nn + 1) * npj], in_=pj[:])
                    nc.sync.dma_start(
                        out=out_d[t * 128:(t + 1) * 128, :], in_=osb[:])

    nc.finalize()
    return nc


def _make_masks():
    kk = np.arange(128)[:, None]
    qq = np.arange(512)[None, :]
    return np.stack(
        [(qq >= 128 * d + kk).astype(np.float32) for d in range(4)], axis=0)


def make_in_maps(x, W_qkv, W_proj):
    """Host-side sharding of full inputs into per-core input maps."""
    x = np.asarray(x, dtype=np.float32)
    W_qkv = np.asarray(W_qkv, dtype=np.float32)
    W_proj = np.asarray(W_proj, dtype=np.float32)
    masks = _make_masks()
    in_maps = []
    for core in range(NCORES):
        b, g = core // GROUPS, core % GROUPS
        in_maps.append({
            "xT": np.ascontiguousarray(x[b].T),
            "wq": np.ascontiguousarray(W_qkv[:, g * FPC:(g + 1) * FPC]),
            "wk": np.ascontiguousarray(W_qkv[:, C + g * FPC:C + (g + 1) * FPC]),
            "wv": np.ascontiguousarray(W_qkv[:, 2 * C + g * FPC:2 * C + (g + 1) * FPC]),
            "wp": np.ascontiguousarray(W_proj[g * FPC:(g + 1) * FPC, :]),
            "masks": masks,
            "ones64": np.ones((1, 64), np.float32),
            "onesv": np.ones((128, 8, 1), np.float32),
        })
    return in_maps


_CACHE = {}


def _get_nc():
    if "nc" not in _CACHE:
        _CACHE["nc"] = build_nc()
    return _CACHE["nc"]


def run_cores(in_maps):
    res = run_bass_kernel_spmd(_get_nc(), in_maps, list(range(NCORES)))
    return res.results


def kernel(x, W_qkv, W_proj):
    results = run_cores(make_in_maps(x, W_qkv, W_proj))
    out = np.empty((B, T, C), dtype=np.float32)
    for b in range(B):
        out[b] = results[GROUPS * b]["out"]
        for g in range(1, GROUPS):
            out[b] += results[GROUPS * b + g]["out"]
    return out
